# revision 1
# baseline (speedup 1.0000x reference)
"""Trainium2 SPMD kernel for edge-wise GNN message passing.

Computes, for each edge e=(s,d):
    out[e] = edge_val[e] * sigmoid(exp(||relu(Eu[s] @ W1.T + b1) - relu(Ev[d] @ W2.T + b2)||_2))

Strategy (8 NeuronCores, edge-parallel):
  - Host: shard 600k edges 8-ways; per core sort edges into 16 (u-bank, v-bank)
    groups (banks of 32768 rows so bank-local node ids fit the int16 indices of
    the GPSIMD dma_gather instruction), pad each group to a multiple of 512.
  - Host: pre-cast Eu/Ev to bf16 (halves gather traffic; distances only feed a
    fully saturated sigmoid(exp(.)), so bf16 is far inside tolerance).
  - Device, per 512-edge segment:
      dma_gather(transpose=True) pulls the 128-dim bf16 rows for the segment's
      edges directly in [k, e] layout (PE-ready moving operand, no on-chip
      transpose);  matmul(lhsT=W.T) -> psum [j, e];  ScalarE fused bias+relu
      psum->sbuf bf16;  VectorE sub + square;  per-128-edge ones-matmul reduces
      over j -> dist^2 [e, 1] in psum;  ScalarE sqrt/exp/sigmoid chain and
      VectorE multiply by edge_val on 512-wide blocks; DMA out.
  - Host: invert the edge permutation, drop padding slots.
"""

import sys
for _p in ("/opt/trn_rl_repo", "/opt/pypackages"):
    if _p not in sys.path:
        sys.path.append(_p)

from contextlib import ExitStack

import ml_dtypes
import numpy as np

import concourse.bass as bass
import concourse.bacc as bacc
import concourse.tile as tile
from concourse import mybir
from concourse.bass_utils import run_bass_kernel_spmd
from concourse.library_config import mlp as mlp_library

F32 = mybir.dt.float32
BF16 = mybir.dt.bfloat16
I16 = mybir.dt.int16
AF = mybir.ActivationFunctionType

N_U, N_V, E, D = 100000, 100000, 600000, 128
NCORES = 8
EPC = E // NCORES            # 75000 edges per core
BANK = 32768                 # rows per gather bank (int16 index range)
NBANKS = (N_U + BANK - 1) // BANK   # 4
SEG = 512                    # edges per compute segment (psum width)
GSEG = 512                   # edges per dma_gather instruction (multiple of SEG)
SINGLE_PACKET = True         # required False when GSEG > 512
ENGINE_SORT = False
SUPER = 128                  # segments per output superblock (= 1 psum bank)


def _bank_rows(b: int, n: int) -> int:
    return min(BANK, n - b * BANK)


# ---------------------------------------------------------------- device code

def _build_program(seg_banks: list[tuple[int, int]]):
    nseg = len(seg_banks)
    T = nseg * SEG

    nc = bacc.Bacc("TRN2", target_bir_lowering=False, debug=False,
                   num_devices=NCORES, num_swdge_queues=4)

    eu_d = nc.dram_tensor("eu", [N_U, D], BF16, kind="ExternalInput")
    ev_d = nc.dram_tensor("ev", [N_V, D], BF16, kind="ExternalInput")
    w1t_d = nc.dram_tensor("w1t", [D, D], BF16, kind="ExternalInput")
    w2t_d = nc.dram_tensor("w2t", [D, D], BF16, kind="ExternalInput")
    b1_d = nc.dram_tensor("b1", [D, 1], F32, kind="ExternalInput")
    b2_d = nc.dram_tensor("b2", [D, 1], F32, kind="ExternalInput")
    ones_d = nc.dram_tensor("ones", [D, 1], BF16, kind="ExternalInput")
    uidx_d = nc.dram_tensor("uidx", [128, T // 16], I16, kind="ExternalInput")
    vidx_d = nc.dram_tensor("vidx", [128, T // 16], I16, kind="ExternalInput")
    evd_d = nc.dram_tensor("evd", [128, T // 128], F32, kind="ExternalInput")
    out_d = nc.dram_tensor("out", [128, T // 128], F32, kind="ExternalOutput")

    with tile.TileContext(nc) as tc, ExitStack() as ctx:
        nc.gpsimd.load_library(mlp_library)

        const = ctx.enter_context(tc.tile_pool(name="const", bufs=1))
        w1t = const.tile([D, D], BF16, tag="w1t")
        nc.sync.dma_start(w1t[:], w1t_d[:])
        w2t = const.tile([D, D], BF16, tag="w2t")
        nc.sync.dma_start(w2t[:], w2t_d[:])
        b1s = const.tile([D, 1], F32, tag="b1s")
        nc.sync.dma_start(b1s[:], b1_d[:])
        b2s = const.tile([D, 1], F32, tag="b2s")
        nc.sync.dma_start(b2s[:], b2_d[:])
        ones = const.tile([D, 1], BF16, tag="ones")
        nc.sync.dma_start(ones[:], ones_d[:])
        uidx = const.tile([128, T // 16], I16, tag="uidx")
        nc.sync.dma_start(uidx[:], uidx_d[:])
        vidx = const.tile([128, T // 16], I16, tag="vidx")
        nc.sync.dma_start(vidx[:], vidx_d[:])
        evs = const.tile([128, T // 128], F32, tag="evs")
        nc.sync.dma_start(evs[:], evd_d[:])

        nreg = nc.gpsimd.to_reg(GSEG)

        # bank views of the embedding tables (row-contiguous APs)
        eu_banks = [eu_d[b * BANK: b * BANK + _bank_rows(b, N_U), :]
                    for b in range(NBANKS)]
        ev_banks = [ev_d[b * BANK: b * BANK + _bank_rows(b, N_V), :]
                    for b in range(NBANKS)]

        gath = ctx.enter_context(tc.tile_pool(name="gath", bufs=4))
        work = ctx.enter_context(tc.tile_pool(name="work", bufs=3))
        pp = ctx.enter_context(tc.tile_pool(name="pp", bufs=3, space="PSUM"))
        dpp = ctx.enter_context(tc.tile_pool(name="dpp", bufs=2, space="PSUM"))
        outp = ctx.enter_context(tc.tile_pool(name="outp", bufs=2))

        for sb_start in range(0, nseg, SUPER):
            sb_seg = min(SUPER, nseg - sb_start)
            fdim = sb_seg * (SEG // 128)
            dist_ps = dpp.tile([128, fdim], F32, tag="dist")
            for sl in range(sb_seg):
                s = sb_start + sl
                ub, vb = seg_banks[s]
                spc = GSEG // SEG  # compute segments per gather chunk
                if s % spc == 0:
                    c = s // spc
                    icols = slice(c * (GSEG // 16), (c + 1) * (GSEG // 16))
                    gut = gath.tile([128, 1, GSEG], BF16, tag="gut")
                    nc.gpsimd.dma_gather(gut[:], eu_banks[ub], uidx[:, icols],
                                         GSEG, nreg, D, transpose=True,
                                         queue_num=(2 * c) % 4,
                                         single_packet=SINGLE_PACKET)
                    gvt = gath.tile([128, 1, GSEG], BF16, tag="gvt")
                    nc.gpsimd.dma_gather(gvt[:], ev_banks[vb], vidx[:, icols],
                                         GSEG, nreg, D, transpose=True,
                                         queue_num=(2 * c + 1) % 4,
                                         single_packet=SINGLE_PACKET)
                    cur_gut, cur_gvt = gut, gvt
                off = (s % spc) * SEG

                mu = pp.tile([128, SEG], F32, tag="mu")
                nc.tensor.matmul(mu[:], lhsT=w1t[:],
                                 rhs=cur_gut[:, 0, off:off + SEG],
                                 start=True, stop=True)
                mv = pp.tile([128, SEG], F32, tag="mv")
                nc.tensor.matmul(mv[:], lhsT=w2t[:],
                                 rhs=cur_gvt[:, 0, off:off + SEG],
                                 start=True, stop=True)

                tu = work.tile([128, SEG], BF16, tag="tu")
                nc.scalar.activation(tu[:], mu[:], AF.Relu, bias=b1s[:])
                tv = work.tile([128, SEG], BF16, tag="tv")
                nc.scalar.activation(tv[:], mv[:], AF.Relu, bias=b2s[:])

                df = work.tile([128, SEG], BF16, tag="df")
                nc.vector.tensor_sub(df[:], tu[:], tv[:])
                dsq = work.tile([128, SEG], BF16, tag="dsq")
                nc.vector.tensor_mul(dsq[:], df[:], df[:])

                for i in range(SEG // 128):
                    c = sl * (SEG // 128) + i
                    nc.tensor.matmul(dist_ps[:, c:c + 1],
                                     lhsT=dsq[:, i * 128:(i + 1) * 128],
                                     rhs=ones[:], start=True, stop=True)

            ocols = slice(sb_start * (SEG // 128),
                          sb_start * (SEG // 128) + fdim)
            dsr = outp.tile([128, fdim], F32, tag="dsr")
            nc.scalar.activation(dsr[:], dist_ps[:], AF.Sqrt)
            ex = outp.tile([128, fdim], F32, tag="ex")
            nc.scalar.activation(ex[:], dsr[:], AF.Exp)
            sg = outp.tile([128, fdim], F32, tag="sg")
            nc.scalar.activation(sg[:], ex[:], AF.Sigmoid)
            ot = outp.tile([128, fdim], F32, tag="ot")
            nc.vector.tensor_mul(ot[:], sg[:], evs[:, ocols])
            nc.sync.dma_start(out_d[:, ocols], ot[:])

    nc.compile()
    return nc


_PROGRAM_CACHE: dict = {}


def _get_program(seg_banks):
    key = tuple(seg_banks)
    if key not in _PROGRAM_CACHE:
        _PROGRAM_CACHE[key] = _build_program(list(seg_banks))
    return _PROGRAM_CACHE[key]


# ------------------------------------------------------------------ host code

def _prepare(Eu, Ev, W1, b1, W2, b2, edge_index, edge_val):
    """Shard + sort edges, build per-core device arrays."""
    src = np.asarray(edge_index[0], dtype=np.int64)
    dst = np.asarray(edge_index[1], dtype=np.int64)
    edge_val = np.asarray(edge_val, dtype=np.float32)

    per_core = []
    counts = np.zeros((NCORES, NBANKS * NBANKS), dtype=np.int64)
    for c in range(NCORES):
        lo, hi = c * EPC, (c + 1) * EPC
        s, d = src[lo:hi], dst[lo:hi]
        g = (s >> 15) * NBANKS + (d >> 15)
        order = np.lexsort((s, g))          # by group, then by u for locality
        counts[c] = np.bincount(g, minlength=NBANKS * NBANKS)
        per_core.append((s, d, edge_val[lo:hi], g, order, lo))

    caps = counts.max(axis=0)
    caps = (caps + GSEG - 1) // GSEG * GSEG   # per-group padded capacity
    group_off = np.concatenate([[0], np.cumsum(caps)]).astype(np.int64)
    T = int(caps.sum())

    seg_banks = []
    for g in range(NBANKS * NBANKS):
        seg_banks.extend([(g // NBANKS, g % NBANKS)] * int(caps[g] // SEG))
    assert len(seg_banks) * SEG == T

    in_maps, origs = [], []
    Eu_bf = np.ascontiguousarray(Eu).astype(ml_dtypes.bfloat16)
    Ev_bf = np.ascontiguousarray(Ev).astype(ml_dtypes.bfloat16)
    w1t = np.ascontiguousarray(np.asarray(W1).T).astype(ml_dtypes.bfloat16)
    w2t = np.ascontiguousarray(np.asarray(W2).T).astype(ml_dtypes.bfloat16)
    b1c = np.ascontiguousarray(np.asarray(b1, dtype=np.float32).reshape(D, 1))
    b2c = np.ascontiguousarray(np.asarray(b2, dtype=np.float32).reshape(D, 1))
    ones = np.ones((D, 1), dtype=ml_dtypes.bfloat16)

    for c in range(NCORES):
        s, d, ev, g, order, lo = per_core[c]
        gs = g[order]
        within = np.arange(EPC, dtype=np.int64) - np.searchsorted(gs, gs)
        slot = group_off[gs] + within
        if ENGINE_SORT:
            # remap within each GSEG gather window so DMA engine e (serving
            # window positions p % 16 == e) sees an ascending address stream:
            # sorted-rank p -> position (p % (GSEG//16)) * 16 + p // (GSEG//16)
            p = slot % GSEG
            rpe = GSEG // 16
            slot = (slot - p) + (p % rpe) * 16 + p // rpe

        u_slots = np.zeros(T, dtype=np.int16)
        v_slots = np.zeros(T, dtype=np.int16)
        ev_slots = np.zeros(T, dtype=np.float32)
        orig = np.full(T, -1, dtype=np.int64)

        u_slots[slot] = (s[order] & (BANK - 1)).astype(np.int16)
        v_slots[slot] = (d[order] & (BANK - 1)).astype(np.int16)
        ev_slots[slot] = ev[order]
        orig[slot] = lo + order

        uidx = np.zeros((128, T // 16), dtype=np.int16)
        uidx[:16] = u_slots.reshape(-1, 16).T
        vidx = np.zeros((128, T // 16), dtype=np.int16)
        vidx[:16] = v_slots.reshape(-1, 16).T
        evd = np.ascontiguousarray(ev_slots.reshape(-1, 128).T)

        in_maps.append({
            "eu": Eu_bf, "ev": Ev_bf, "w1t": w1t, "w2t": w2t,
            "b1": b1c, "b2": b2c, "ones": ones,
            "uidx": uidx, "vidx": vidx, "evd": evd,
        })
        origs.append(orig)

    return seg_banks, in_maps, origs


def _run(inputs: dict, trace: bool = False):
    seg_banks, in_maps, origs = _prepare(**inputs)
    nc = _get_program(seg_banks)
    bkr = run_bass_kernel_spmd(nc, in_maps, core_ids=list(range(NCORES)),
                               trace=trace)
    out_full = np.zeros(E, dtype=np.float32)
    for c in range(NCORES):
        arr = np.asarray(bkr.results[c]["out"], dtype=np.float32)
        slots = np.ascontiguousarray(arr.T).reshape(-1)
        orig = origs[c]
        m = orig >= 0
        out_full[orig[m]] = slots[m]
    return out_full, bkr


def kernel(**inputs) -> np.ndarray:
    out, _ = _run(inputs, trace=False)
    return out

